# revision 4
# baseline (speedup 1.0000x reference)
"""Trainium2 Bass kernel for the Community Convolution layer.

Computation (see module docstring in the reference):
  stage 1: per-subject [16,16] community affinity update -> ratio2[b]
           (ratio with diagonal forced to 1.0)
  stage 2: per-(b,k) GCN update  Hp_k = beta * D^-1/2 (Wp*Rn) D^-1/2 Hp theta
  stage 3: W_out[b,i,j] = W[b,i,j] * ratio2[b, memb[i], memb[j]]

Sharding (8 cores, identical program, different data):
  - stage 3: W row-sharded; core c handles rows [c*384,(c+1)*384) of every
    subject.  fac is computed as one-hot matmuls:
        fac = (ratio2^T @ E_own)^T @ E_full
    where E[c,m] = (memb[m]==c), built on-device with iota + is_equal.
  - stage 2: the 160 (b,k) pairs are sharded 20 per core.  Wp/Rn/Hp are
    host pre-transposed (pure layout) so no device transposes are needed.
  - stage 1: tiny; replicated on all cores, computed fully on device.
"""

import sys

for _p in ("/opt/trn_rl_repo", "/root/.axon_site/_ro/trn_rl_repo"):
    if _p not in sys.path:
        sys.path.insert(0, _p)

import numpy as np

import concourse.bacc as bacc
import concourse.mybir as mybir
from concourse.bass import AP  # noqa: F401
from concourse.masks import make_identity
from concourse.tile import TileContext
from concourse.bass_utils import run_bass_kernel_spmd

# problem constants (hardcoded; kernel.py must be self-contained)
B, K, NP, N, FD = 10, 16, 192, 3072, 70
ALPHA, BETA = 0.1, 0.1
NCORES = 8
ROWS = N // NCORES            # 384 rows of W per core per subject
RT = ROWS // 128              # 3 row tiles per subject
PAIRS = (B * K) // NCORES     # 20 gcn pairs per core
CHUNK = 512                   # fac column chunk (one PSUM bank, fp32)
NCH = N // CHUNK              # 6 chunks per row tile

F32 = mybir.dt.float32
F32R = mybir.dt.float32r
I32 = mybir.dt.int32

# fp32r runs the PE at 1 cycle/row (vs 4 for fp32) for free-dim >= 256.
# The fac operands are one-hot selections so only input rounding matters.
USE_F32R_FAC = True


# dtype for fac-matmul operands: producers round to fp32r so the PE can use
# the 1-cycle/row path (vs 4 for fp32)
FACDT = F32R if USE_F32R_FAC else F32


def build_kernel():
    nc = bacc.Bacc("TRN2")

    # ---- device I/O ----
    w_in = nc.dram_tensor("w_in", [B, ROWS, N], F32, kind="ExternalInput")
    memb_full = nc.dram_tensor("memb_full", [B, N], I32, kind="ExternalInput")
    memb_own = nc.dram_tensor("memb_own", [B, ROWS], I32, kind="ExternalInput")
    hc_in = nc.dram_tensor("hc", [B, K, K], F32, kind="ExternalInput")
    rc_in = nc.dram_tensor("rc", [B, K, K], F32, kind="ExternalInput")
    rcsd_in = nc.dram_tensor("rcsd", [B, K], F32, kind="ExternalInput")
    wpt_in = nc.dram_tensor("wpt", [PAIRS, NP, NP], F32, kind="ExternalInput")
    rnt_in = nc.dram_tensor("rnt", [PAIRS, NP, NP], F32, kind="ExternalInput")
    hpt_in = nc.dram_tensor("hpt", [PAIRS, FD, NP], F32, kind="ExternalInput")
    deg_in = nc.dram_tensor("degp", [PAIRS, NP], F32, kind="ExternalInput")
    theta_in = nc.dram_tensor("theta", [FD, FD], F32, kind="ExternalInput")

    w_out = nc.dram_tensor("w_out", [B, ROWS, N], F32, kind="ExternalOutput")
    hpk_out = nc.dram_tensor("hpk", [PAIRS, NP, FD], F32, kind="ExternalOutput")

    with TileContext(nc) as tc:
        with (
            tc.tile_pool(name="const", bufs=1) as cpool,
            tc.tile_pool(name="persist", bufs=B) as ppool,       # per-subject live tiles
            tc.tile_pool(name="s1", bufs=2) as s1pool,           # stage-1 scratch (sbuf)
            tc.tile_pool(name="s1ps", bufs=1, space="PSUM") as s1ps,
            tc.tile_pool(name="wtile", bufs=4) as wpool,         # big W stream tiles
            tc.tile_pool(name="et", bufs=2) as etpool,
            tc.tile_pool(name="facps", bufs=3, space="PSUM") as facps,
            tc.tile_pool(name="gcn", bufs=2) as gpool,
            tc.tile_pool(name="gcnps", bufs=1, space="PSUM") as gps,
        ):
            # ---- constants ----
            ident = cpool.tile([K, K], F32, tag="ident")
            make_identity(nc, ident[:, :])
            ones_row = cpool.tile([1, K], F32, tag="ones_row")
            nc.vector.memset(ones_row[:, :], 1.0)
            iota16 = cpool.tile([K, 1], F32, tag="iota16")
            nc.gpsimd.iota(iota16[:, :], pattern=[[0, 1]], base=0, channel_multiplier=1,
                           allow_small_or_imprecise_dtypes=True)
            theta_sb = cpool.tile([FD, FD], F32, tag="theta")
            nc.sync.dma_start(out=theta_sb[:, :], in_=theta_in[:, :])

            # ---- stage 1: ratio2[b] and R_own[b] for all subjects ----
            ratio2 = []
            r_own = []
            for b in range(B):
                hc_sb = s1pool.tile([K, K], F32, tag="hc")
                rc_sb = s1pool.tile([K, K], F32, tag="rcs1")
                rcsd_sb = s1pool.tile([1, K], F32, tag="rcsd")
                nc.sync.dma_start(out=hc_sb[:, :], in_=hc_in[b])
                nc.sync.dma_start(out=rc_sb[:, :], in_=rc_in[b])
                nc.sync.dma_start(out=rcsd_sb[:, :], in_=rcsd_in[b : b + 1, :])

                # column sums of Hc (as a [K,1] column): transpose then reduce
                hcT_ps = s1ps.tile([K, K], F32, tag="s1")
                nc.tensor.transpose(hcT_ps[:, :], hc_sb[:, :], ident[:, :])
                hcT_sb = s1pool.tile([K, K], F32, tag="hcT")
                nc.scalar.copy(hcT_sb[:, :], hcT_ps[:, :])
                csum = s1pool.tile([K, 1], F32, tag="csum")
                nc.vector.tensor_reduce(
                    csum[:, :], hcT_sb[:, :], axis=mybir.AxisListType.X,
                    op=mybir.AluOpType.add,
                )
                # del_diag (row [1,K]) = colsum(Hc) @ Rc
                dd_ps = s1ps.tile([1, K], F32, tag="s1")
                nc.tensor.matmul(dd_ps[:, :], csum[:, :], rc_sb[:, :],
                                 start=True, stop=True)
                # ds = alpha * del_diag / diag(Rcs)
                recr = s1pool.tile([1, K], F32, tag="recr")
                nc.vector.reciprocal(recr[:, :], rcsd_sb[:, :])
                ds_sb = s1pool.tile([1, K], F32, tag="ds")
                nc.vector.scalar_tensor_tensor(
                    ds_sb[:, :], dd_ps[:, :], ALPHA, recr[:, :],
                    op0=mybir.AluOpType.mult, op1=mybir.AluOpType.mult,
                )
                # broadcast ds across partitions: ones^T @ ds
                dsb_ps = s1ps.tile([K, K], F32, tag="s1")
                nc.tensor.matmul(dsb_ps[:, :], ones_row[:, :], ds_sb[:, :],
                                 start=True, stop=True)
                del_rc = s1pool.tile([K, K], F32, tag="delrc")
                nc.vector.tensor_mul(del_rc[:, :], rc_sb[:, :], dsb_ps[:, :])
                delT_ps = s1ps.tile([K, K], F32, tag="s1")
                nc.tensor.transpose(delT_ps[:, :], del_rc[:, :], ident[:, :])
                rec_rc = s1pool.tile([K, K], F32, tag="recrc")
                nc.vector.reciprocal(rec_rc[:, :], rc_sb[:, :])
                sum2 = s1pool.tile([K, K], F32, tag="sum2")
                nc.vector.tensor_add(sum2[:, :], del_rc[:, :], delT_ps[:, :])
                tmat = s1pool.tile([K, K], F32, tag="tmat")
                nc.vector.tensor_mul(tmat[:, :], sum2[:, :], rec_rc[:, :])
                # zero the diagonal ((ratio-1) has diag forced so ratio2 diag==1)
                t2 = s1pool.tile([K, K], F32, tag="t2")
                nc.gpsimd.affine_select(
                    out=t2[:, :], in_=tmat[:, :],
                    compare_op=mybir.AluOpType.not_equal, fill=0.0,
                    base=0, pattern=[[-1, K]], channel_multiplier=1,
                )
                r2 = ppool.tile([K, K], FACDT, tag="ratio2")
                nc.scalar.add(r2[:, :], t2[:, :], 1.0)
                ratio2.append(r2)

                # R_own[b] = ratio2^T @ E_own  ([K, ROWS])
                mo_bc = s1pool.tile([K, ROWS], I32, tag="mo_bc")
                nc.sync.dma_start(
                    out=mo_bc[:, :], in_=memb_own[b].partition_broadcast(K)
                )
                et_own = s1pool.tile([K, ROWS], FACDT, tag="et_own")
                nc.vector.tensor_scalar(
                    et_own[:, :], mo_bc[:, :], iota16[:, :], None,
                    op0=mybir.AluOpType.is_equal,
                )
                ro_ps = s1ps.tile([K, ROWS], F32, tag="s1")
                nc.tensor.matmul(
                    ro_ps[:, :],
                    r2[:, :],
                    et_own[:, :],
                    start=True, stop=True,
                )
                ro = ppool.tile([K, ROWS], FACDT, tag="r_own")
                nc.scalar.copy(ro[:, :], ro_ps[:, :])
                r_own.append(ro)

            # ---- main loop: per subject W scaling (+ interleaved GCN pairs) ----
            def gcn_pair(p):
                wt1 = gpool.tile([128, NP], F32, tag="wt1")
                wt2 = gpool.tile([64, NP], F32, tag="wt2")
                rt1 = gpool.tile([128, NP], F32, tag="rt1")
                rt2 = gpool.tile([64, NP], F32, tag="rt2")
                ht = gpool.tile([FD, NP], F32, tag="ht")
                nc.sync.dma_start(out=wt1[:, :], in_=wpt_in[p, 0:128, :])
                nc.sync.dma_start(out=wt2[:, :], in_=wpt_in[p, 128:NP, :])
                nc.sync.dma_start(out=rt1[:, :], in_=rnt_in[p, 0:128, :])
                nc.sync.dma_start(out=rt2[:, :], in_=rnt_in[p, 128:NP, :])
                nc.sync.dma_start(out=ht[:, :], in_=hpt_in[p])
                dg1 = gpool.tile([128, 1], F32, tag="dg1")
                dg2 = gpool.tile([64, 1], F32, tag="dg2")
                nc.sync.dma_start(out=dg1[:, :], in_=deg_in[p, 0:128].unsqueeze(1))
                nc.sync.dma_start(out=dg2[:, :], in_=deg_in[p, 128:NP].unsqueeze(1))
                rc1 = gpool.tile([128, 1], F32, tag="rc1")
                rc2 = gpool.tile([64, 1], F32, tag="rc2")
                nc.vector.reciprocal(rc1[:, :], dg1[:, :])
                nc.vector.reciprocal(rc2[:, :], dg2[:, :])
                d1 = gpool.tile([128, 1], F32, tag="d1")
                d2 = gpool.tile([64, 1], F32, tag="d2")
                nc.scalar.sqrt(d1[:, :], rc1[:, :])
                nc.scalar.sqrt(d2[:, :], rc2[:, :])
                # S = (Wp*Rn)^T chunks (m on partitions)
                s1 = gpool.tile([128, NP], F32, tag="s1g")
                s2 = gpool.tile([64, NP], F32, tag="s2g")
                nc.vector.tensor_mul(s1[:, :], wt1[:, :], rt1[:, :])
                nc.vector.tensor_mul(s2[:, :], wt2[:, :], rt2[:, :])
                # Y = Hp @ theta  (rows m on partitions)
                y1_ps = gps.tile([128, FD], F32, tag="y1")
                y2_ps = gps.tile([64, FD], F32, tag="y2")
                nc.tensor.matmul(y1_ps[:, :], ht[:, 0:128], theta_sb[:, :],
                                 start=True, stop=True)
                nc.tensor.matmul(y2_ps[:, :], ht[:, 128:NP], theta_sb[:, :],
                                 start=True, stop=True)
                yd1 = gpool.tile([128, FD], F32, tag="yd1")
                yd2 = gpool.tile([64, FD], F32, tag="yd2")
                nc.vector.tensor_scalar(
                    yd1[:, :], y1_ps[:, :], d1[:, :], None, op0=mybir.AluOpType.mult
                )
                nc.vector.tensor_scalar(
                    yd2[:, :], y2_ps[:, :], d2[:, :], None, op0=mybir.AluOpType.mult
                )
                # out rows n: o[n,g] = sum_m S^T[m,n] Yd[m,g]
                o1_ps = gps.tile([128, FD], F32, tag="o1")
                o2_ps = gps.tile([64, FD], F32, tag="o2")
                nc.tensor.matmul(o1_ps[:, :], s1[:, 0:128], yd1[:, :],
                                 start=True, stop=False)
                nc.tensor.matmul(o1_ps[:, :], s2[:, 0:128], yd2[:, :],
                                 start=False, stop=True)
                nc.tensor.matmul(o2_ps[:, :], s1[:, 128:NP], yd1[:, :],
                                 start=True, stop=False)
                nc.tensor.matmul(o2_ps[:, :], s2[:, 128:NP], yd2[:, :],
                                 start=False, stop=True)
                h1 = gpool.tile([128, FD], F32, tag="h1")
                h2 = gpool.tile([64, FD], F32, tag="h2")
                nc.vector.tensor_scalar(
                    h1[:, :], o1_ps[:, :], d1[:, :], BETA,
                    op0=mybir.AluOpType.mult, op1=mybir.AluOpType.mult,
                )
                nc.vector.tensor_scalar(
                    h2[:, :], o2_ps[:, :], d2[:, :], BETA,
                    op0=mybir.AluOpType.mult, op1=mybir.AluOpType.mult,
                )
                nc.sync.dma_start(out=hpk_out[p, 0:128, :], in_=h1[:, :])
                nc.sync.dma_start(out=hpk_out[p, 128:NP, :], in_=h2[:, :])

            for b in range(B):
                # build E_full[b]  [K, N]
                mb_bc = etpool.tile([K, N], I32, tag="mb_bc")
                nc.sync.dma_start(
                    out=mb_bc[:, :], in_=memb_full[b].partition_broadcast(K)
                )
                et = etpool.tile([K, N], FACDT, tag="et")
                nc.vector.tensor_scalar(
                    et[:, :], mb_bc[:, :], iota16[:, :], None,
                    op0=mybir.AluOpType.is_equal,
                )
                for t in range(RT):
                    wt = wpool.tile([128, N], F32, tag="w")
                    nc.sync.dma_start(
                        out=wt[:, :], in_=w_in[b, t * 128 : (t + 1) * 128, :]
                    )
                    for j in range(NCH):
                        fac_ps = facps.tile([128, CHUNK], F32, tag="fac")
                        nc.tensor.matmul(
                            fac_ps[:, :],
                            r_own[b][:, t * 128 : (t + 1) * 128],
                            et[:, j * CHUNK : (j + 1) * CHUNK],
                            start=True, stop=True,
                        )
                        nc.vector.tensor_mul(
                            wt[:, j * CHUNK : (j + 1) * CHUNK],
                            wt[:, j * CHUNK : (j + 1) * CHUNK],
                            fac_ps[:, :],
                        )
                    nc.sync.dma_start(
                        out=w_out[b, t * 128 : (t + 1) * 128, :], in_=wt[:, :]
                    )
                # interleave 2 gcn pairs per subject
                for p in range(2 * b, min(2 * b + 2, PAIRS)):
                    gcn_pair(p)

    nc.finalize()
    return nc


_NC_CACHE = None


def _get_nc():
    global _NC_CACHE
    if _NC_CACHE is None:
        _NC_CACHE = build_kernel()
    return _NC_CACHE


def _prep_in_maps(inputs):
    Hc = np.ascontiguousarray(inputs["Hc"], dtype=np.float32)
    Rc = np.ascontiguousarray(inputs["Rc"], dtype=np.float32)
    rcsd = np.ascontiguousarray(
        np.diagonal(np.asarray(inputs["rcs_diag"], dtype=np.float32), axis1=-2, axis2=-1)
    )
    Wp = np.asarray(inputs["Wp"], dtype=np.float32).reshape(B * K, NP, NP)
    Rn = np.asarray(inputs["Rn"], dtype=np.float32).reshape(B * K, NP, NP)
    Hp = np.asarray(inputs["Hp"], dtype=np.float32).reshape(B * K, NP, FD)
    WpT = np.ascontiguousarray(Wp.transpose(0, 2, 1))
    RnT = np.ascontiguousarray(Rn.transpose(0, 2, 1))
    HpT = np.ascontiguousarray(Hp.transpose(0, 2, 1))
    deg = np.asarray(inputs["deg"], dtype=np.float32).reshape(B * K, NP)
    W = np.asarray(inputs["W"], dtype=np.float32)
    theta = np.ascontiguousarray(inputs["theta"], dtype=np.float32)
    memb = np.ascontiguousarray(np.asarray(inputs["memb"], dtype=np.int32))

    in_maps = []
    for c in range(NCORES):
        r0, r1 = c * ROWS, (c + 1) * ROWS
        p0, p1 = c * PAIRS, (c + 1) * PAIRS
        in_maps.append(
            {
                "w_in": np.ascontiguousarray(W[:, r0:r1, :]),
                "memb_full": memb,
                "memb_own": np.ascontiguousarray(memb[:, r0:r1]),
                "hc": Hc,
                "rc": Rc,
                "rcsd": rcsd,
                "wpt": WpT[p0:p1],
                "rnt": RnT[p0:p1],
                "hpt": HpT[p0:p1],
                "degp": np.ascontiguousarray(deg[p0:p1]),
                "theta": theta,
            }
        )
    return in_maps


def _assemble(results):
    W_out = np.concatenate([r["w_out"] for r in results], axis=1)
    Hpk = np.concatenate([r["hpk"] for r in results], axis=0).reshape(B, K, NP, FD)
    return W_out, Hpk


def run(inputs, **kw):
    nc = _get_nc()
    in_maps = _prep_in_maps(inputs)
    res = run_bass_kernel_spmd(nc, in_maps, core_ids=list(range(NCORES)), **kw)
    return res


def kernel(**inputs):
    res = run(inputs)
    return _assemble(res.results)


# revision 5
# speedup vs baseline: 1.1926x; 1.1926x over previous
"""Trainium2 Bass kernel for the Community Convolution layer.

Computation (see module docstring in the reference):
  stage 1: per-subject [16,16] community affinity update -> ratio2[b]
           (ratio with diagonal forced to 1.0)
  stage 2: per-(b,k) GCN update  Hp_k = beta * D^-1/2 (Wp*Rn) D^-1/2 Hp theta
  stage 3: W_out[b,i,j] = W[b,i,j] * ratio2[b, memb[i], memb[j]]

Sharding (8 cores, identical program, different data):
  - stage 3: W row-sharded; core c handles rows [c*384,(c+1)*384) of every
    subject.  fac is computed as one-hot matmuls:
        fac = (ratio2^T @ E_own)^T @ E_full
    where E[c,m] = (memb[m]==c), built on-device with iota + is_equal.
  - stage 2: the 160 (b,k) pairs are sharded 20 per core.  Wp/Rn/Hp are
    host pre-transposed (pure layout) so no device transposes are needed.
  - stage 1: tiny; replicated on all cores, computed fully on device.
"""

import sys

for _p in ("/opt/trn_rl_repo", "/root/.axon_site/_ro/trn_rl_repo"):
    if _p not in sys.path:
        sys.path.insert(0, _p)

import numpy as np

import concourse.bacc as bacc
import concourse.mybir as mybir
from concourse.bass import AP  # noqa: F401
from concourse.masks import make_identity
from concourse.tile import TileContext
from concourse.bass_utils import run_bass_kernel_spmd

# problem constants (hardcoded; kernel.py must be self-contained)
B, K, NP, N, FD = 10, 16, 192, 3072, 70
ALPHA, BETA = 0.1, 0.1
NCORES = 8
ROWS = N // NCORES            # 384 rows of W per core per subject
RT = ROWS // 128              # 3 row tiles per subject
PAIRS = (B * K) // NCORES     # 20 gcn pairs per core
CHUNK = 512                   # fac column chunk (one PSUM bank, fp32)
NCH = N // CHUNK              # 6 chunks per row tile

F32 = mybir.dt.float32
F32R = mybir.dt.float32r
I32 = mybir.dt.int32

# The fac matmuls run in fp16 with a hi/lo two-pass split of ratio2:
# fp16 streams at 1 cycle/row on the PE (vs 4 for fp32) with cheap weight
# loads, and hi+lo recovers ~22 mantissa bits (operands are one-hot
# selections, accumulation is fp32 in PSUM).
F16 = mybir.dt.float16


def build_kernel():
    nc = bacc.Bacc("TRN2")

    # ---- device I/O ----
    w_in = nc.dram_tensor("w_in", [B, ROWS, N], F32, kind="ExternalInput")
    memb_full = nc.dram_tensor("memb_full", [B, N], I32, kind="ExternalInput")
    memb_own = nc.dram_tensor("memb_own", [B, ROWS], I32, kind="ExternalInput")
    hc_in = nc.dram_tensor("hc", [B, K, K], F32, kind="ExternalInput")
    rc_in = nc.dram_tensor("rc", [B, K, K], F32, kind="ExternalInput")
    rcsd_in = nc.dram_tensor("rcsd", [B, K], F32, kind="ExternalInput")
    wpt_in = nc.dram_tensor("wpt", [PAIRS, NP, NP], F32, kind="ExternalInput")
    rnt_in = nc.dram_tensor("rnt", [PAIRS, NP, NP], F32, kind="ExternalInput")
    hpt_in = nc.dram_tensor("hpt", [PAIRS, FD, NP], F32, kind="ExternalInput")
    deg_in = nc.dram_tensor("degp", [PAIRS, NP], F32, kind="ExternalInput")
    theta_in = nc.dram_tensor("theta", [FD, FD], F32, kind="ExternalInput")

    w_out = nc.dram_tensor("w_out", [B, ROWS, N], F32, kind="ExternalOutput")
    hpk_out = nc.dram_tensor("hpk", [PAIRS, NP, FD], F32, kind="ExternalOutput")

    with TileContext(nc) as tc:
        with (
            tc.tile_pool(name="const", bufs=1) as cpool,
            tc.tile_pool(name="persist", bufs=B) as ppool,       # per-subject live tiles
            tc.tile_pool(name="s1", bufs=2) as s1pool,           # stage-1 scratch (sbuf)
            tc.tile_pool(name="s1ps", bufs=1, space="PSUM") as s1ps,
            tc.tile_pool(name="wtile", bufs=2) as wpool,         # big W stream tiles
            tc.tile_pool(name="et", bufs=2) as etpool,
            tc.tile_pool(name="facps", bufs=3, space="PSUM") as facps,
            tc.tile_pool(name="gcn", bufs=2) as gpool,
            tc.tile_pool(name="gcnps", bufs=1, space="PSUM") as gps,
        ):
            # ---- constants ----
            ident = cpool.tile([K, K], F32, tag="ident")
            make_identity(nc, ident[:, :])
            ones_row = cpool.tile([1, K], F32, tag="ones_row")
            nc.vector.memset(ones_row[:, :], 1.0)
            iota16 = cpool.tile([K, 1], F32, tag="iota16")
            nc.gpsimd.iota(iota16[:, :], pattern=[[0, 1]], base=0, channel_multiplier=1,
                           allow_small_or_imprecise_dtypes=True)
            theta_sb = cpool.tile([FD, FD], F32, tag="theta")
            nc.sync.dma_start(out=theta_sb[:, :], in_=theta_in[:, :])

            # ---- stage 1: ratio2[b] and R_own[b] for all subjects ----
            ratio2 = []
            r_own = []
            for b in range(B):
                hc_sb = s1pool.tile([K, K], F32, tag="hc")
                rc_sb = s1pool.tile([K, K], F32, tag="rcs1")
                rcsd_sb = s1pool.tile([1, K], F32, tag="rcsd")
                nc.gpsimd.dma_start(out=hc_sb[:, :], in_=hc_in[b])
                nc.gpsimd.dma_start(out=rc_sb[:, :], in_=rc_in[b])
                nc.gpsimd.dma_start(out=rcsd_sb[:, :], in_=rcsd_in[b : b + 1, :])

                # column sums of Hc (as a [K,1] column): transpose then reduce
                hcT_ps = s1ps.tile([K, K], F32, tag="s1")
                nc.tensor.transpose(hcT_ps[:, :], hc_sb[:, :], ident[:, :])
                hcT_sb = s1pool.tile([K, K], F32, tag="hcT")
                nc.scalar.copy(hcT_sb[:, :], hcT_ps[:, :])
                csum = s1pool.tile([K, 1], F32, tag="csum")
                nc.vector.tensor_reduce(
                    csum[:, :], hcT_sb[:, :], axis=mybir.AxisListType.X,
                    op=mybir.AluOpType.add,
                )
                # del_diag (row [1,K]) = colsum(Hc) @ Rc
                dd_ps = s1ps.tile([1, K], F32, tag="s1")
                nc.tensor.matmul(dd_ps[:, :], csum[:, :], rc_sb[:, :],
                                 start=True, stop=True)
                # ds = alpha * del_diag / diag(Rcs)
                recr = s1pool.tile([1, K], F32, tag="recr")
                nc.vector.reciprocal(recr[:, :], rcsd_sb[:, :])
                ds_sb = s1pool.tile([1, K], F32, tag="ds")
                nc.vector.scalar_tensor_tensor(
                    ds_sb[:, :], dd_ps[:, :], ALPHA, recr[:, :],
                    op0=mybir.AluOpType.mult, op1=mybir.AluOpType.mult,
                )
                # broadcast ds across partitions: ones^T @ ds
                dsb_ps = s1ps.tile([K, K], F32, tag="s1")
                nc.tensor.matmul(dsb_ps[:, :], ones_row[:, :], ds_sb[:, :],
                                 start=True, stop=True)
                del_rc = s1pool.tile([K, K], F32, tag="delrc")
                nc.vector.tensor_mul(del_rc[:, :], rc_sb[:, :], dsb_ps[:, :])
                delT_ps = s1ps.tile([K, K], F32, tag="s1")
                nc.tensor.transpose(delT_ps[:, :], del_rc[:, :], ident[:, :])
                rec_rc = s1pool.tile([K, K], F32, tag="recrc")
                nc.vector.reciprocal(rec_rc[:, :], rc_sb[:, :])
                sum2 = s1pool.tile([K, K], F32, tag="sum2")
                nc.vector.tensor_add(sum2[:, :], del_rc[:, :], delT_ps[:, :])
                tmat = s1pool.tile([K, K], F32, tag="tmat")
                nc.vector.tensor_mul(tmat[:, :], sum2[:, :], rec_rc[:, :])
                # zero the diagonal ((ratio-1) has diag forced so ratio2 diag==1)
                t2 = s1pool.tile([K, K], F32, tag="t2")
                nc.gpsimd.affine_select(
                    out=t2[:, :], in_=tmat[:, :],
                    compare_op=mybir.AluOpType.not_equal, fill=0.0,
                    base=0, pattern=[[-1, K]], channel_multiplier=1,
                )
                r2 = ppool.tile([K, K], F32, tag="ratio2")
                nc.scalar.add(r2[:, :], t2[:, :], 1.0)
                ratio2.append(r2)

                # split ratio2 into fp16 hi + lo parts
                r_hi = s1pool.tile([K, K], F16, tag="r_hi")
                nc.vector.tensor_copy(r_hi[:, :], r2[:, :])
                r_dif = s1pool.tile([K, K], F32, tag="r_dif")
                nc.vector.tensor_sub(r_dif[:, :], r2[:, :], r_hi[:, :])
                r_lo = s1pool.tile([K, K], F16, tag="r_lo")
                nc.vector.tensor_copy(r_lo[:, :], r_dif[:, :])

                # R_own[b] = ratio2^T @ E_own  ([K, ROWS]), hi/lo parts
                mo_bc = s1pool.tile([K, ROWS], I32, tag="mo_bc")
                nc.gpsimd.dma_start(
                    out=mo_bc[:, :], in_=memb_own[b].partition_broadcast(K)
                )
                et_own = s1pool.tile([K, ROWS], F16, tag="et_own")
                nc.vector.tensor_scalar(
                    et_own[:, :], mo_bc[:, :], iota16[:, :], None,
                    op0=mybir.AluOpType.is_equal,
                )
                ros = []
                for part in (r_hi, r_lo):
                    ro_ps = s1ps.tile([K, ROWS], F32, tag="s1")
                    nc.tensor.matmul(ro_ps[:, :], part[:, :], et_own[:, :],
                                     start=True, stop=True)
                    ro = ppool.tile([K, ROWS], F16,
                                    tag="r_own_hi" if part is r_hi else "r_own_lo")
                    nc.scalar.copy(ro[:, :], ro_ps[:, :])
                    ros.append(ro)
                r_own.append(ros)

            # ---- main loop: per subject W scaling (+ interleaved GCN pairs) ----
            def gcn_pair(p):
                wt1 = gpool.tile([128, NP], F32, tag="wt1")
                wt2 = gpool.tile([64, NP], F32, tag="wt2")
                rt1 = gpool.tile([128, NP], F32, tag="rt1")
                rt2 = gpool.tile([64, NP], F32, tag="rt2")
                ht = gpool.tile([FD, NP], F32, tag="ht")
                nc.scalar.dma_start(out=wt1[:, :], in_=wpt_in[p, 0:128, :])
                nc.scalar.dma_start(out=wt2[:, :], in_=wpt_in[p, 128:NP, :])
                nc.scalar.dma_start(out=rt1[:, :], in_=rnt_in[p, 0:128, :])
                nc.scalar.dma_start(out=rt2[:, :], in_=rnt_in[p, 128:NP, :])
                nc.scalar.dma_start(out=ht[:, :], in_=hpt_in[p])
                dg1 = gpool.tile([128, 1], F32, tag="dg1")
                dg2 = gpool.tile([64, 1], F32, tag="dg2")
                nc.scalar.dma_start(out=dg1[:, :], in_=deg_in[p, 0:128].unsqueeze(1))
                nc.scalar.dma_start(out=dg2[:, :], in_=deg_in[p, 128:NP].unsqueeze(1))
                rc1 = gpool.tile([128, 1], F32, tag="rc1")
                rc2 = gpool.tile([64, 1], F32, tag="rc2")
                nc.vector.reciprocal(rc1[:, :], dg1[:, :])
                nc.vector.reciprocal(rc2[:, :], dg2[:, :])
                d1 = gpool.tile([128, 1], F32, tag="d1")
                d2 = gpool.tile([64, 1], F32, tag="d2")
                nc.scalar.sqrt(d1[:, :], rc1[:, :])
                nc.scalar.sqrt(d2[:, :], rc2[:, :])
                # S = (Wp*Rn)^T chunks (m on partitions)
                s1 = gpool.tile([128, NP], F32, tag="s1g")
                s2 = gpool.tile([64, NP], F32, tag="s2g")
                nc.gpsimd.tensor_mul(s1[:, :], wt1[:, :], rt1[:, :])
                nc.gpsimd.tensor_mul(s2[:, :], wt2[:, :], rt2[:, :])
                # Y = Hp @ theta  (rows m on partitions)
                y1_ps = gps.tile([128, FD], F32, tag="y1")
                y2_ps = gps.tile([64, FD], F32, tag="y2")
                nc.tensor.matmul(y1_ps[:, :], ht[:, 0:128], theta_sb[:, :],
                                 start=True, stop=True)
                nc.tensor.matmul(y2_ps[:, :], ht[:, 128:NP], theta_sb[:, :],
                                 start=True, stop=True)
                yd1 = gpool.tile([128, FD], F32, tag="yd1")
                yd2 = gpool.tile([64, FD], F32, tag="yd2")
                nc.vector.tensor_scalar(
                    yd1[:, :], y1_ps[:, :], d1[:, :], None, op0=mybir.AluOpType.mult
                )
                nc.vector.tensor_scalar(
                    yd2[:, :], y2_ps[:, :], d2[:, :], None, op0=mybir.AluOpType.mult
                )
                # out rows n: o[n,g] = sum_m S^T[m,n] Yd[m,g]
                o1_ps = gps.tile([128, FD], F32, tag="o1")
                o2_ps = gps.tile([64, FD], F32, tag="o2")
                nc.tensor.matmul(o1_ps[:, :], s1[:, 0:128], yd1[:, :],
                                 start=True, stop=False)
                nc.tensor.matmul(o1_ps[:, :], s2[:, 0:128], yd2[:, :],
                                 start=False, stop=True)
                nc.tensor.matmul(o2_ps[:, :], s1[:, 128:NP], yd1[:, :],
                                 start=True, stop=False)
                nc.tensor.matmul(o2_ps[:, :], s2[:, 128:NP], yd2[:, :],
                                 start=False, stop=True)
                h1 = gpool.tile([128, FD], F32, tag="h1")
                h2 = gpool.tile([64, FD], F32, tag="h2")
                nc.vector.tensor_scalar(
                    h1[:, :], o1_ps[:, :], d1[:, :], BETA,
                    op0=mybir.AluOpType.mult, op1=mybir.AluOpType.mult,
                )
                nc.vector.tensor_scalar(
                    h2[:, :], o2_ps[:, :], d2[:, :], BETA,
                    op0=mybir.AluOpType.mult, op1=mybir.AluOpType.mult,
                )
                nc.scalar.dma_start(out=hpk_out[p, 0:128, :], in_=h1[:, :])
                nc.scalar.dma_start(out=hpk_out[p, 128:NP, :], in_=h2[:, :])

            for b in range(B):
                # build E_full[b]  [K, N]
                mb_bc = etpool.tile([K, N], I32, tag="mb_bc")
                nc.gpsimd.dma_start(
                    out=mb_bc[:, :], in_=memb_full[b].partition_broadcast(K)
                )
                et = etpool.tile([K, N], F16, tag="et")
                nc.vector.tensor_scalar(
                    et[:, :], mb_bc[:, :], iota16[:, :], None,
                    op0=mybir.AluOpType.is_equal,
                )
                # one big load for the whole subject stripe: [128, RT, N]
                wt = wpool.tile([128, RT, N], F32, tag="w")
                nc.sync.dma_start(
                    out=wt[:, :, :],
                    in_=w_in[b].rearrange("(t p) n -> p t n", p=128),
                )
                ro_hi, ro_lo = r_own[b]
                for t in range(RT):
                    for j in range(NCH):
                        fac_ps = facps.tile([128, CHUNK], F32, tag="fac")
                        nc.tensor.matmul(
                            fac_ps[:, :],
                            ro_hi[:, t * 128 : (t + 1) * 128],
                            et[:, j * CHUNK : (j + 1) * CHUNK],
                            start=True, stop=False,
                        )
                        nc.tensor.matmul(
                            fac_ps[:, :],
                            ro_lo[:, t * 128 : (t + 1) * 128],
                            et[:, j * CHUNK : (j + 1) * CHUNK],
                            start=False, stop=True,
                        )
                        nc.vector.tensor_mul(
                            wt[:, t, j * CHUNK : (j + 1) * CHUNK],
                            wt[:, t, j * CHUNK : (j + 1) * CHUNK],
                            fac_ps[:, :],
                        )
                    nc.sync.dma_start(
                        out=w_out[b, t * 128 : (t + 1) * 128, :], in_=wt[:, t, :]
                    )
                # interleave 2 gcn pairs per subject
                for p in range(2 * b, min(2 * b + 2, PAIRS)):
                    gcn_pair(p)

    nc.finalize()
    return nc


_NC_CACHE = None


def _get_nc():
    global _NC_CACHE
    if _NC_CACHE is None:
        _NC_CACHE = build_kernel()
    return _NC_CACHE


def _prep_in_maps(inputs):
    Hc = np.ascontiguousarray(inputs["Hc"], dtype=np.float32)
    Rc = np.ascontiguousarray(inputs["Rc"], dtype=np.float32)
    rcsd = np.ascontiguousarray(
        np.diagonal(np.asarray(inputs["rcs_diag"], dtype=np.float32), axis1=-2, axis2=-1)
    )
    Wp = np.asarray(inputs["Wp"], dtype=np.float32).reshape(B * K, NP, NP)
    Rn = np.asarray(inputs["Rn"], dtype=np.float32).reshape(B * K, NP, NP)
    Hp = np.asarray(inputs["Hp"], dtype=np.float32).reshape(B * K, NP, FD)
    WpT = np.ascontiguousarray(Wp.transpose(0, 2, 1))
    RnT = np.ascontiguousarray(Rn.transpose(0, 2, 1))
    HpT = np.ascontiguousarray(Hp.transpose(0, 2, 1))
    deg = np.asarray(inputs["deg"], dtype=np.float32).reshape(B * K, NP)
    W = np.asarray(inputs["W"], dtype=np.float32)
    theta = np.ascontiguousarray(inputs["theta"], dtype=np.float32)
    memb = np.ascontiguousarray(np.asarray(inputs["memb"], dtype=np.int32))

    in_maps = []
    for c in range(NCORES):
        r0, r1 = c * ROWS, (c + 1) * ROWS
        p0, p1 = c * PAIRS, (c + 1) * PAIRS
        in_maps.append(
            {
                "w_in": np.ascontiguousarray(W[:, r0:r1, :]),
                "memb_full": memb,
                "memb_own": np.ascontiguousarray(memb[:, r0:r1]),
                "hc": Hc,
                "rc": Rc,
                "rcsd": rcsd,
                "wpt": WpT[p0:p1],
                "rnt": RnT[p0:p1],
                "hpt": HpT[p0:p1],
                "degp": np.ascontiguousarray(deg[p0:p1]),
                "theta": theta,
            }
        )
    return in_maps


def _assemble(results):
    W_out = np.concatenate([r["w_out"] for r in results], axis=1)
    Hpk = np.concatenate([r["hpk"] for r in results], axis=0).reshape(B, K, NP, FD)
    return W_out, Hpk


def run(inputs, **kw):
    nc = _get_nc()
    in_maps = _prep_in_maps(inputs)
    res = run_bass_kernel_spmd(nc, in_maps, core_ids=list(range(NCORES)), **kw)
    return res


def kernel(**inputs):
    res = run(inputs)
    return _assemble(res.results)
